# revision 22
# baseline (speedup 1.0000x reference)
"""BiLevelGAT Trainium2 kernel: 8-core SPMD, users row-sharded 512/core.

Self-contained: hardcodes shapes from the problem spec
  N_U = N_I = 4096, D_IN = D = 256, D_QOL = 4, B = 8192.

kernel(**inputs) takes the FULL inputs (as produced by setup_inputs()) and
returns (r_hat [8192], alpha_s [4096,4096], alpha_c [4096,4096]) float32,
matching the reference return tuple.

Design (per core k, owning users U0=512k .. U0+512):
  host folds:  w1s = user_W@soc_W@soc_a[:D],  w2s = user_W@soc_W@soc_a[D:]
               w1c = user_W@cnt_W@cnt_a[:D],  w2c = item_W@cnt_W@cnt_a[D:]
               Wts = user_W@soc_W, Wtc = item_W@cnt_W (value matrices)
  device:      a2 rows via PE (w2 broadcast-lhsT trick), a1 cols via PE,
               masked softmax = one DVE scalar_tensor_tensor (adj*BIG+a2) +
               ACT Prelu(bias=a1-BIG, slope .2) + ACT Exp(accum_out=denom) +
               DVE normalize; PE block-transposes of alpha -> msg matmuls
               against Wts/Wtc values; ELU + QoL gate; pair prediction in
               transposed space via one-hot gather matmul.
  host:        reassembles alpha rows, scatters per-core pair results.
"""
import sys, os

for _p in ("/opt/trn_rl_repo", "/root/.axon_site", "/root/.axon_site/_ro/trn_rl_repo",
           "/root/.axon_site/_ro/pypackages"):
    if os.path.isdir(_p) and _p not in sys.path:
        sys.path.append(_p)

import numpy as np
import concourse.bass as bass
import concourse.tile as tile
import concourse.mybir as mybir
from concourse import bacc
from concourse.bass_utils import run_bass_kernel_spmd

dt = mybir.dt
AF = mybir.ActivationFunctionType
ALU = mybir.AluOpType

N_CORES = 8
N = 4096          # users == items
D = 256
U = 512           # users per core
RT = 4            # 128-row tiles per core
CT = 32           # 128-col tiles
PAIR_CAP = 1536
BIG = 600.0       # mask offset; 0.2*600 = 120 > 104 so exp(masked) == +0.0

f32, f32r, i32, i8 = dt.float32, dt.float32r, dt.int32, dt.int8

_CACHE = {}
_ABL = 0


def _flat(ap):
    """[p, a, b] view -> [p, a*b]."""
    return ap.rearrange("p a b -> p (a b)")


def _build(cs_const: float, cc_const: float, pred_b2: float, repeat: int = 1):
    nc = bacc.Bacc("TRN2", target_bir_lowering=False)

    def din(name, shape, dty=f32r):
        return nc.dram_tensor(name, shape, dty, kind="ExternalInput")

    xuT = din("xuT", [256, N])
    xiT = din("xiT", [256, N])
    xuT_own = din("xuT_own", [256, U])
    user_W = din("user_W", [256, D])
    item_W = din("item_W", [256, D])
    Wts = din("Wts", [256, D])
    Wtc = din("Wtc", [256, D])
    w2s_b = din("w2s_b", [256, 128])
    w2c_b = din("w2c_b", [256, 128])
    w1s = din("w1s", [128, 2])
    w1c = din("w1c", [128, 2])
    quT_a = din("quT_a", [5, U])
    gate_Wa = din("gate_Wa", [5, D])
    qproj_Wa = din("qproj_Wa", [5, D])
    pred_W1 = din("pred_W1", [512, D])
    pred_W2 = din("pred_W2", [128, 2])
    pred_b1c = din("pred_b1c", [128, 2], f32)
    onehot = din("onehot", [512, PAIR_CAP])
    xi_gT = din("xi_gT", [256, PAIR_CAP])
    ident = din("ident", [128, 128], f32)
    adj_s = din("adj_s", [U, N], i8)
    adj_c = din("adj_c", [U, N], i8)

    alpha_s_o = nc.dram_tensor("alpha_s_o", [U, N], f32, kind="ExternalOutput")
    alpha_c_o = nc.dram_tensor("alpha_c_o", [U, N], f32, kind="ExternalOutput")
    rhat_o = nc.dram_tensor("rhat_o", [1, PAIR_CAP], f32, kind="ExternalOutput")

    def ldk(pool, src, k_tiles, ncols, dty=f32r):
        t = pool.tile([128, k_tiles, ncols], dty, name=f"t_{src.name}")
        nc.sync.dma_start(out=t, in_=src[:].rearrange("(k p) n -> p k n", p=128))
        return t

    with tile.TileContext(nc) as tc:
      for _rep in range(repeat):
        with tc.tile_pool(name="const", bufs=1) as const:
            # mid pool: tensors alive only through the attention loop
            # (a2 rows + value matrices); explicitly released before the
            # pair phase to make room.
            _mid_cm = tc.tile_pool(name="mid", bufs=1)
            mid = _mid_cm.__enter__()
            # ======== load + h-phase (xuT/xiT freed afterwards) ========
            with tc.tile_pool(name="xload", bufs=1) as xl:
                # halves as separate DMAs so the a2/h matmuls start sooner
                t_xuT = xl.tile([128, 2, N], f32r, name="t_xuT")
                t_xiT = xl.tile([128, 2, N], f32r, name="t_xiT")
                for t, src in ((t_xuT, xuT), (t_xiT, xiT)):
                    v = src[:].rearrange("(k p) n -> p k n", p=128)
                    nc.sync.dma_start(out=t[:, :, :N // 2], in_=v[:, :, :N // 2])
                    nc.sync.dma_start(out=t[:, :, N // 2:], in_=v[:, :, N // 2:])
                t_xuT_own = ldk(const, xuT_own, 2, U)
                t_userW = ldk(const, user_W, 2, D)
                t_itemW = ldk(const, item_W, 2, D)
                t_Wts = ldk(const, Wts, 2, D)
                t_Wtc = ldk(const, Wtc, 2, D)
                t_w2sb = ldk(const, w2s_b, 2, 128)
                t_w2cb = ldk(const, w2c_b, 2, 128)
                t_w1s = const.tile([128, 2], f32r)
                nc.sync.dma_start(out=t_w1s, in_=w1s[:])
                t_w1c = const.tile([128, 2], f32r)
                nc.sync.dma_start(out=t_w1c, in_=w1c[:])
                t_quT = const.tile([5, U], f32r)
                nc.sync.dma_start(out=t_quT, in_=quT_a[:])
                t_gateW = const.tile([5, D], f32r)
                nc.sync.dma_start(out=t_gateW, in_=gate_Wa[:])
                t_qprojW = const.tile([5, D], f32r)
                nc.sync.dma_start(out=t_qprojW, in_=qproj_Wa[:])
                t_ident = const.tile([128, 128], f32)
                nc.sync.dma_start(out=t_ident, in_=ident[:])

                # ---- a2 broadcast rows [128, N] per attention ----
                t_a2sb = mid.tile([128, N], f32)
                t_a2cb = mid.tile([128, N], f32)
                with tc.tile_pool(name="psA", bufs=1, space="PSUM") as pa:
                    for w2b, xT, dst in ((t_w2sb, t_xuT, t_a2sb),
                                         (t_w2cb, t_xiT, t_a2cb)):
                        for half in range(2):
                            ps_ab = pa.tile([128, 2048], f32, tag="ab",
                                            name="ps_ab")
                            for ch in range(4):
                                c0 = half * 2048 + ch * 512
                                for k in range(2):
                                    nc.tensor.matmul(
                                        ps_ab[:, ch * 512:(ch + 1) * 512],
                                        w2b[:, k, :], xT[:, k, c0:c0 + 512],
                                        start=(k == 0), stop=(k == 1))
                            nc.scalar.activation(
                                out=dst[:, half * 2048:(half + 1) * 2048],
                                in_=ps_ab, func=AF.Copy)

                # ---- value matrices Wh_s / Wi_c [N, D] (c-major) ----
                t_WhS = mid.tile([128, CT, D], f32r)
                t_WiC = mid.tile([128, CT, D], f32r)
                t_hu_own = const.tile([128, RT, D], f32)
                t_a1s = const.tile([128, RT], f32)
                t_a1c = const.tile([128, RT], f32)
                with tc.tile_pool(name="psH", bufs=2, space="PSUM") as ph:
                    for w, xT, dst in ((t_Wts, t_xuT, t_WhS),
                                       (t_Wtc, t_xiT, t_WiC)):
                        for g in range(8):
                            ps_h = ph.tile([128, 1024], f32, tag="h",
                                           name="ps_h")
                            for m in range(4):
                                mt = g * 4 + m
                                for k in range(2):
                                    nc.tensor.matmul(
                                        ps_h[:, m * 256:(m + 1) * 256],
                                        xT[:, k, mt * 128:(mt + 1) * 128],
                                        w[:, k, :],
                                        start=(k == 0), stop=(k == 1))
                            o = _flat(dst[:, g * 4:(g + 1) * 4, :])
                            if g % 4 == 0:
                                nc.scalar.activation(out=o, in_=ps_h,
                                                     func=AF.Copy)
                            else:
                                nc.vector.tensor_copy(out=o, in_=ps_h)
                    # h_u own rows
                    ps_hu = ph.tile([128, 1024], f32, tag="h", name="ps_hu")
                    for rt in range(RT):
                        for k in range(2):
                            nc.tensor.matmul(
                                ps_hu[:, rt * 256:(rt + 1) * 256],
                                t_xuT_own[:, k, rt * 128:(rt + 1) * 128],
                                t_userW[:, k, :],
                                start=(k == 0), stop=(k == 1))
                    nc.scalar.activation(out=_flat(t_hu_own), in_=ps_hu,
                                         func=AF.Copy)
                    # a1 columns (+ prelu const folded in)
                    for w1, dst, cst in ((t_w1s, t_a1s, cs_const),
                                         (t_w1c, t_a1c, cc_const)):
                        for rt in range(RT):
                            ps_a1 = ph.tile([128, 1], f32, tag="a1",
                                            name="ps_a1")
                            for k in range(2):
                                # M=1/N=1 violates fp32r ISA rules; use fp32
                                nc.tensor.matmul(
                                    ps_a1,
                                    t_xuT_own[:, k, rt * 128:(rt + 1) * 128]
                                    .bitcast(f32),
                                    w1[:, k:k + 1].bitcast(f32),
                                    start=(k == 0), stop=(k == 1))
                            nc.vector.tensor_scalar(
                                out=dst[:, rt:rt + 1], in0=ps_a1,
                                scalar1=float(cst), scalar2=None, op0=ALU.add)

            # ======== attention loop: 4 row tiles x 2 attentions ========
            t_elu = const.tile([128, RT, D], f32)
            with tc.tile_pool(name="att", bufs=2) as ap_, \
                 tc.tile_pool(name="psT", bufs=2, space="PSUM") as pt, \
                 tc.tile_pool(name="psM", bufs=2, space="PSUM") as pm:
                # SW-pipelined emission: stage A (mask+prelu+exp) runs
                # LAG jobs ahead of stage B (transpose+msg+normalize+out) so
                # no engine's in-order stream blocks on another engine.
                jobs = [(rt, att) for rt in range(RT) for att in range(2)]
                LAG = 2
                st = {}
                rt_msgs = {rt: {} for rt in range(RT)}

                def stage_a(jid):
                    rt, att = jobs[jid]
                    adj_d = adj_s if att == 0 else adj_c
                    a2b = t_a2sb if att == 0 else t_a2cb
                    a1col = t_a1s if att == 0 else t_a1c
                    t_adj = ap_.tile([128, N], i8, tag="adj", name="t_adj",
                                     bufs=3)
                    nc.sync.dma_start(
                        out=t_adj, in_=adj_d[rt * 128:(rt + 1) * 128, :])
                    t_t = ap_.tile([128, N], f32, tag="t", name="t_t", bufs=3)
                    nc.vector.scalar_tensor_tensor(
                        out=t_t, in0=t_adj, scalar=BIG, in1=a2b,
                        op0=ALU.mult, op1=ALU.add)
                    nc.scalar.activation(out=t_t, in_=t_t, func=AF.Prelu,
                                         bias=a1col[:, rt:rt + 1],
                                         scale=1.0, alpha=0.2)
                    t_den = ap_.tile([128, 1], f32, tag="den", name="t_den",
                                     bufs=4)
                    nc.scalar.activation(out=t_t, in_=t_t, func=AF.Exp,
                                         accum_out=t_den)
                    t_rd = ap_.tile([128, 1], f32, tag="rd", name="t_rd",
                                    bufs=4)
                    # guard: empty rows (denom==0) must yield alpha==0, not NaN
                    nc.vector.tensor_scalar(out=t_den, in0=t_den,
                                            scalar1=1e-30, scalar2=None,
                                            op0=ALU.max)
                    nc.vector.reciprocal(out=t_rd, in_=t_den)
                    st[jid] = (t_t, t_rd)

                def stage_b(jid):
                    rt, att = jobs[jid]
                    alpha_d = alpha_s_o if att == 0 else alpha_c_o
                    vals = t_WhS if att == 0 else t_WiC
                    t_ex, t_rd = st.pop(jid)
                    ps_msg = pm.tile([128, D], f32, tag=f"msg{att}",
                                     name="ps_msg")
                    for ch in range(4):
                        ps_tr = pt.tile([128, 1024], f32, tag="tr",
                                        name="ps_tr")
                        for b in (range(8) if _ABL < 2 else [0]):
                            ct = ch * 8 + b
                            nc.tensor.transpose(
                                ps_tr[:, b * 128:(b + 1) * 128],
                                t_ex[:, ct * 128:(ct + 1) * 128],
                                t_ident)
                        t_aT = ap_.tile([128, 1024], f32r, tag="aT",
                                        name="t_aT")
                        if ch == 0:
                            nc.scalar.activation(out=t_aT, in_=ps_tr,
                                                 func=AF.Copy)
                        else:
                            nc.vector.tensor_copy(out=t_aT, in_=ps_tr)
                        bs = range(8) if _ABL < 1 else ([0] if ch == 0 else [])
                        for b in bs:
                            ct = ch * 8 + b
                            nc.tensor.matmul(
                                ps_msg, t_aT[:, b * 128:(b + 1) * 128],
                                vals[:, ct, :],
                                start=(ct == 0), stop=(ct == CT - 1),
                                skip_group_check=True)
                    # alpha out = ex / denom (after transposes read ex)
                    nc.vector.tensor_scalar(out=t_ex, in0=t_ex,
                                            scalar1=t_rd,
                                            scalar2=None, op0=ALU.mult)
                    nc.sync.dma_start(
                        out=alpha_d[rt * 128:(rt + 1) * 128, :], in_=t_ex)
                    rt_msgs[rt][att] = (ps_msg, t_rd)
                    if att == 1:
                        finish_rt(rt)

                def finish_rt(rt):
                    # t1 = msg_s*rd_s + msg_c*rd_c + h_u_own; elu
                    (ps_s, rd_s), (ps_c, rd_c) = rt_msgs[rt][0], rt_msgs[rt][1]
                    t_v = ap_.tile([128, D], f32, tag="v", name="t_v", bufs=2)
                    nc.vector.tensor_scalar(out=t_v, in0=ps_s, scalar1=rd_s,
                                            scalar2=None, op0=ALU.mult)
                    t_m = ap_.tile([128, D], f32, tag="m", name="t_m", bufs=2)
                    nc.vector.scalar_tensor_tensor(out=t_m, in0=ps_c,
                                                   scalar=rd_c, in1=t_v,
                                                   op0=ALU.mult, op1=ALU.add)
                    nc.vector.tensor_tensor(out=t_m, in0=t_m,
                                            in1=t_hu_own[:, rt, :],
                                            op=ALU.add)
                    t_mn = ap_.tile([128, D], f32, tag="mn", name="t_mn",
                                    bufs=2)
                    nc.vector.tensor_scalar(out=t_mn, in0=t_m, scalar1=0.0,
                                            scalar2=None, op0=ALU.min)
                    nc.scalar.activation(out=t_mn, in_=t_mn, func=AF.Exp)
                    nc.vector.tensor_scalar(out=t_m, in0=t_m, scalar1=0.0,
                                            scalar2=None, op0=ALU.max)
                    nc.vector.scalar_tensor_tensor(
                        out=t_elu[:, rt, :], in0=t_mn, scalar=-1.0, in1=t_m,
                        op0=ALU.add, op1=ALU.add)

                for step in range(len(jobs) + LAG):
                    if step < len(jobs):
                        stage_a(step)
                    if step >= LAG:
                        stage_b(step - LAG)

            _mid_cm.__exit__(None, None, None)
            # ======== QoL gate (sigmoids batched -> one table switch) ====
            t_gate = const.tile([128, RT * D], f32)
            with tc.tile_pool(name="psQ", bufs=1, space="PSUM") as pq:
                ps_gate = pq.tile([128, RT * D], f32, tag="q", name="ps_gate")
                for rt in range(RT):
                    nc.tensor.matmul(ps_gate[:, rt * D:(rt + 1) * D],
                                     t_quT[:, rt * 128:(rt + 1) * 128],
                                     t_gateW, start=True, stop=True)
                nc.scalar.activation(out=t_gate, in_=ps_gate, func=AF.Sigmoid)
                ps_qp = pq.tile([128, RT * D], f32, tag="q", name="ps_qp")
                for rt in range(RT):
                    nc.tensor.matmul(ps_qp[:, rt * D:(rt + 1) * D],
                                     t_quT[:, rt * 128:(rt + 1) * 128],
                                     t_qprojW, start=True, stop=True)
                nc.vector.tensor_tensor(out=t_gate, in0=t_gate, in1=ps_qp,
                                        op=ALU.mult)
            # h_u_out = elu + gate*qproj (alpha_q folded into qproj_Wa)
            t_huo = const.tile([128, RT, D], f32r)
            nc.vector.tensor_tensor(
                out=_flat(t_huo), in0=_flat(t_elu), in1=t_gate, op=ALU.add)

            # ======== pair phase ========
            with tc.tile_pool(name="pair", bufs=1) as pr, \
                 tc.tile_pool(name="psP", bufs=2, space="PSUM") as pp:
                t_oh = ldk(pr, onehot, 4, PAIR_CAP)
                t_xigT = ldk(pr, xi_gT, 2, PAIR_CAP)
                t_pW1 = ldk(pr, pred_W1, 4, D)
                t_pW2 = pr.tile([128, 2], f32r)
                nc.sync.dma_start(out=t_pW2, in_=pred_W2[:])
                t_pb1 = pr.tile([128, 2], f32)
                nc.sync.dma_start(out=t_pb1, in_=pred_b1c[:])

                NCH = PAIR_CAP // 512
                t_catT = pr.tile([128, 4, PAIR_CAP], f32r)
                for m in range(2):                      # user-side d halves
                    for nch in range(NCH):
                        ps_u = pp.tile([128, 512], f32, tag="p", name="ps_u")
                        for k in range(RT):
                            nc.tensor.matmul(
                                ps_u, t_huo[:, k, m * 128:(m + 1) * 128],
                                t_oh[:, k, nch * 512:(nch + 1) * 512],
                                start=(k == 0), stop=(k == RT - 1))
                        nc.scalar.activation(
                            out=t_catT[:, m, nch * 512:(nch + 1) * 512],
                            in_=ps_u, func=AF.Copy)
                for m in range(2):                      # item-side d halves
                    for nch in range(NCH):
                        ps_i = pp.tile([128, 512], f32, tag="p", name="ps_i")
                        for k in range(2):
                            nc.tensor.matmul(
                                ps_i, t_itemW[:, k, m * 128:(m + 1) * 128],
                                t_xigT[:, k, nch * 512:(nch + 1) * 512],
                                start=(k == 0), stop=(k == 1))
                        nc.scalar.activation(
                            out=t_catT[:, 2 + m, nch * 512:(nch + 1) * 512],
                            in_=ps_i, func=AF.Copy)
                t_hidT = pr.tile([128, 2, PAIR_CAP], f32r)
                for m in range(2):
                    for nch in range(NCH):
                        ps_hid = pp.tile([128, 512], f32, tag="p",
                                         name="ps_hid")
                        for k in range(4):
                            nc.tensor.matmul(
                                ps_hid, t_pW1[:, k, m * 128:(m + 1) * 128],
                                t_catT[:, k, nch * 512:(nch + 1) * 512],
                                start=(k == 0), stop=(k == 3))
                        nc.scalar.activation(
                            out=t_hidT[:, m, nch * 512:(nch + 1) * 512],
                            in_=ps_hid, func=AF.Relu, bias=t_pb1[:, m:m + 1])
                ps_r = pp.tile([1, PAIR_CAP], f32, tag="r", name="ps_r",
                               bufs=1)
                for nch in range(NCH):
                    for k in range(2):
                        # M=1 violates fp32r ISA rules; use fp32
                        nc.tensor.matmul(
                            ps_r[:, nch * 512:(nch + 1) * 512],
                            t_pW2[:, k:k + 1].bitcast(f32),
                            t_hidT[:, k, nch * 512:(nch + 1) * 512]
                            .bitcast(f32),
                            start=(k == 0), stop=(k == 1))
                t_sig = pr.tile([1, PAIR_CAP], f32)
                nc.scalar.activation(out=t_sig, in_=ps_r, func=AF.Sigmoid,
                                     bias=float(pred_b2))
                nc.vector.tensor_scalar(out=t_sig, in0=t_sig, scalar1=4.0,
                                        scalar2=1.0, op0=ALU.mult, op1=ALU.add)
                nc.sync.dma_start(out=rhat_o[:], in_=t_sig)

    nc.finalize()
    return nc


def _prep(inputs):
    """Host-side folding + per-core input maps. Pure numpy."""
    g = {k: np.asarray(v) for k, v in inputs.items()}
    f = lambda k: g[k].astype(np.float64)

    soc_W, cnt_W = f("soc_W"), f("cnt_W")
    user_W, item_W = f("user_W"), f("item_W")
    user_b, item_b = f("user_b"), f("item_b")
    soc_a, cnt_a = f("soc_a"), f("cnt_a")

    va1s, va2s = soc_W @ soc_a[:D], soc_W @ soc_a[D:]
    va1c, va2c = cnt_W @ cnt_a[:D], cnt_W @ cnt_a[D:]
    w1s_full = user_W @ va1s
    w2s_full = user_W @ va2s
    w1c_full = user_W @ va1c
    w2c_full = item_W @ va2c
    cs_const = float(user_b @ va1s + user_b @ va2s - BIG)
    cc_const = float(user_b @ va1c + item_b @ va2c - BIG)

    Wts = (user_W @ soc_W).astype(np.float32)
    Wtc = (item_W @ cnt_W).astype(np.float32)

    x_u, x_i = g["x_u"].astype(np.float32), g["x_i"].astype(np.float32)
    q_u = g["q_u"].astype(np.float32)
    user_idx = np.asarray(g["user_idx"], np.int64)
    item_idx = np.asarray(g["item_idx"], np.int64)
    alpha_q = float(g["alpha_q"])

    xuT = np.ascontiguousarray(x_u.T)
    xiT = np.ascontiguousarray(x_i.T)
    w2s_b = np.ascontiguousarray(
        np.repeat(w2s_full.astype(np.float32)[:, None], 128, 1))
    w2c_b = np.ascontiguousarray(
        np.repeat(w2c_full.astype(np.float32)[:, None], 128, 1))
    w1s_sb = np.ascontiguousarray(w1s_full.astype(np.float32).reshape(2, 128).T)
    w1c_sb = np.ascontiguousarray(w1c_full.astype(np.float32).reshape(2, 128).T)

    gate_Wa = np.concatenate(
        [g["gate_W"].astype(np.float32), g["gate_b"].astype(np.float32)[None, :]], 0)
    qproj_Wa = alpha_q * np.concatenate(
        [g["qproj_W"].astype(np.float32), g["qproj_b"].astype(np.float32)[None, :]], 0)
    pred_W1 = g["pred_W1"].astype(np.float32)
    pred_W2v = g["pred_W2"].astype(np.float32)[:, 0]
    pred_W2sb = np.ascontiguousarray(pred_W2v.reshape(2, 128).T)
    pred_b1c = np.ascontiguousarray(
        g["pred_b1"].astype(np.float32).reshape(2, 128).T)
    pred_b2 = float(g["pred_b2"][0])
    ident = np.eye(128, dtype=np.float32)

    shared = dict(xuT=xuT, xiT=xiT, user_W=g["user_W"].astype(np.float32),
                  item_W=g["item_W"].astype(np.float32), Wts=Wts, Wtc=Wtc,
                  w2s_b=w2s_b, w2c_b=w2c_b, w1s=w1s_sb, w1c=w1c_sb,
                  gate_Wa=gate_Wa, qproj_Wa=qproj_Wa, pred_W1=pred_W1,
                  pred_W2=pred_W2sb, pred_b1c=pred_b1c, ident=ident)

    # adjacency is {0,1}: repack to int8 (lossless, 4x less DMA)
    adj_s_full = (np.asarray(g["social_adj"]) != 0).astype(np.int8)
    adj_c_full = (np.asarray(g["ui_adj"]) != 0).astype(np.int8)

    in_maps, sels = [], []
    owner = user_idx // U
    for k in range(N_CORES):
        u0 = k * U
        sel = np.flatnonzero(owner == k)
        assert sel.size <= PAIR_CAP, f"core {k}: {sel.size} pairs > {PAIR_CAP}"
        sels.append(sel)
        oh = np.zeros((U, PAIR_CAP), np.float32)
        oh[user_idx[sel] - u0, np.arange(sel.size)] = 1.0
        xig = np.zeros((D, PAIR_CAP), np.float32)
        xig[:, :sel.size] = x_i[item_idx[sel]].T
        quT_a = np.concatenate(
            [q_u[u0:u0 + U].T, np.ones((1, U), np.float32)], 0)
        m = dict(shared)
        m.update(xuT_own=np.ascontiguousarray(xuT[:, u0:u0 + U]),
                 quT_a=np.ascontiguousarray(quT_a),
                 onehot=oh, xi_gT=xig,
                 adj_s=adj_s_full[u0:u0 + U], adj_c=adj_c_full[u0:u0 + U])
        in_maps.append(m)

    return in_maps, sels, (cs_const, cc_const, pred_b2)


def _run(inputs, trace=False, trace_cores=None):
    in_maps, sels, consts = _prep(inputs)
    key = tuple(np.round(consts, 9))
    if key not in _CACHE:
        _CACHE[key] = _build(*consts)
    nc = _CACHE[key]
    kw = {}
    if trace:
        kw = dict(trace=True, trace_cores=trace_cores or [0])
    res = run_bass_kernel_spmd(nc, in_maps, core_ids=list(range(N_CORES)), **kw)

    alpha_s = np.concatenate([res.results[k]["alpha_s_o"] for k in range(N_CORES)], 0)
    alpha_c = np.concatenate([res.results[k]["alpha_c_o"] for k in range(N_CORES)], 0)
    B = inputs["user_idx"].shape[0]
    r_hat = np.zeros((B,), np.float32)
    for k in range(N_CORES):
        sel = sels[k]
        r_hat[sel] = res.results[k]["rhat_o"][0, :sel.size]
    return (r_hat, alpha_s, alpha_c), res


def kernel(**inputs):
    out, _ = _run(inputs)
    return out
